# revision 1
# baseline (speedup 1.0000x reference)
"""Multi-head attention (12 heads, RoPE, causal SDPA) for Trainium2, 8 cores.

Sharding: batch (2) x head-group (4 groups of 3 heads). Each core computes,
for its (batch b, head-group hg): QKV projection for its 3 heads, RoPE,
causal attention, and a partial out-projection [T, C] restricted to its
heads' rows of w_out. The host sums the 4 head-group partials per batch.

Device-side layouts (T = 2048, C = 768, D = 64 per head):
  xT   [768, 2048]  x[b] transposed (c on partitions)
  wA   [768, 640]   packed lhsT weights: cols 0:128 [q0|q1], 128:256 [k0|k1],
                    256:320 q2, 320:384 k2, 384:576 w_v (3 heads), 64 zero pad
  wo   [64, 2304]   w_out rows for this head-group: 3 x [64 d, 768 c]
  cosT/sinT [128, 2048]  RoPE tables transposed, stacked twice (64 d x 2)
  rT   [128, 128]   rotate_half as matmul lhsT: rot(q)T_chunk = rT.T @ qT_chunk
  tri  [128, 128]   tri[kr, qc] = 1 if qc >= kr (causal keep-mask, S^T layout)

Attention is computed transposed (S^T[k, q] = K Q^T blocks) so that softmax
P^T lands in [k, q] layout, which feeds P@V directly with v in natural [t, d]
layout (no transposes). Softmax has no max-subtraction (scores are O(1) by
construction) and the denominator comes from an all-ones column appended to
the stationary v operand. Normalization is applied in the [d, q] layout via a
K=1 outer-product broadcast of 1/denominator. Matmuls use float32r (~13
mantissa bits, 4x faster than fp32 on the PE).
"""
import numpy as np

B, T, C, H, D = 2, 2048, 768, 12, 64
HPG = 3                    # heads per group
NG = B * (H // HPG)        # 8 cores
ROPE_BASE = 10000.0
TQ = T // 128              # 16 t-tiles
NCC = C // 128             # 6 contraction chunks
GW = 1024                  # attention q-group width
NGRP = T // GW             # 2 q-groups

_CACHE = {}


def _build_nc(reps=1):
    from concourse import bacc, tile, mybir

    f32 = mybir.dt.float32
    f32r = mybir.dt.float32r
    Exp = mybir.ActivationFunctionType.Exp
    mult = mybir.AluOpType.mult
    add = mybir.AluOpType.add

    nc = bacc.Bacc("TRN2", target_bir_lowering=False, debug=False,
                   num_devices=NG)

    xT_d = nc.dram_tensor("xT", [C, T], f32r, kind="ExternalInput").ap()
    wA_d = nc.dram_tensor("wA", [C, 640], f32r, kind="ExternalInput").ap()
    woA_d = nc.dram_tensor("woA", [2 * D, C], f32r, kind="ExternalInput").ap()
    woB_d = nc.dram_tensor("woB", [D, C], f32r, kind="ExternalInput").ap()
    cosT_d = nc.dram_tensor("cosT", [128, T], f32, kind="ExternalInput").ap()
    sinT_d = nc.dram_tensor("sinT", [128, T], f32, kind="ExternalInput").ap()
    rT_d = nc.dram_tensor("rT", [128, 128], f32r, kind="ExternalInput").ap()
    tri_d = nc.dram_tensor("tri", [128, 128], f32, kind="ExternalInput").ap()
    out_d = nc.dram_tensor("out", [T, C], f32, kind="ExternalOutput").ap()

    with tile.TileContext(nc) as tc:
      for rep in range(reps):
        with tc.tile_pool(name=f"persist{rep}", bufs=1) as pp:
                dmaq = [nc.sync, nc.scalar, nc.gpsimd]

                # ---- persistent constants ----
                woA = pp.tile([2 * D, C], f32r, tag="woA")
                dmaq[1].dma_start(woA[:], woA_d[:])
                woB = pp.tile([D, C], f32r, tag="woB")
                dmaq[1].dma_start(woB[:], woB_d[:])
                tri = pp.tile([128, 128], f32, tag="tri")
                dmaq[2].dma_start(tri[:], tri_d[:])
                onesf = pp.tile([1, D], f32, tag="onesf")
                nc.vector.memset(onesf[:], 1.0)
                ones = pp.tile([1, D], f32r, tag="ones")
                nc.scalar.copy(ones[:], onesf[:])

                # persistent intermediates: [q0|q1], [k0|k1], [q2], [k2]
                # (projection computes [q2|k2] packed; RoPE splits into two
                # 64-row tiles via cross-partition DVE writes)
                qk_rows = [128, 128, 64, 64]
                qkT = [pp.tile([qk_rows[m], T], f32r, tag=f"qkT{m}",
                               name=f"qkT{m}") for m in range(4)]
                v_sb = pp.tile([128, TQ, HPG, 65], f32r, tag="v_sb")
                onesw = pp.tile([128, TQ * HPG], f32, tag="onesw")
                nc.vector.memset(onesw[:], 1.0)
                nc.scalar.copy(
                    v_sb[:, :, :, 64:65],
                    onesw[:].rearrange("p (a b) -> p a b", b=HPG).rearrange(
                        "p a b -> p a b ()"))
                attnT_A = pp.tile([2 * D, T], f32r, tag="attnTA")
                attnT_B = pp.tile([D, T], f32r, tag="attnTB")
                attn_dst = [(attnT_A, 0), (attnT_A, D), (attnT_B, 0)]

                # ================= QKV phase (scoped pools) =================
                qkv_pool = tc.tile_pool(name=f"qkv{rep}", bufs=1)
                qp = qkv_pool.__enter__()
                qkv_ps_pool = tc.tile_pool(name=f"qkvps{rep}", bufs=8, space="PSUM")
                qps = qkv_ps_pool.__enter__()

                xT = [qp.tile([128, T], f32r, tag=f"xT{c}", name=f"xT{c}")
                      for c in range(NCC)]
                wA = [qp.tile([128, 640], f32r, tag=f"wA{c}", name=f"wA{c}")
                      for c in range(NCC)]
                # weights first (small), then xT column-major in [128, 512]
                # pieces so the first projection chunk's deps arrive in ~2us
                for c in range(NCC):
                    dmaq[c % 3].dma_start(
                        wA[c][:], wA_d[128 * c:128 * (c + 1), :])
                qi = 0
                for n in range(4):
                    for c in range(NCC):
                        nsl = slice(512 * n, 512 * (n + 1))
                        dmaq[qi % 3].dma_start(
                            xT[c][:, nsl], xT_d[128 * c:128 * (c + 1), nsl])
                        qi += 1
                cosT = qp.tile([128, T], f32, tag="cosT")
                sinT = qp.tile([128, T], f32, tag="sinT")
                dmaq[2].dma_start(cosT[:], cosT_d[:])
                dmaq[0].dma_start(sinT[:], sinT_d[:])
                rT = qp.tile([128, 128], f32r, tag="rT")
                dmaq[1].dma_start(rT[:], rT_d[:])

                # q/k projection + RoPE; rot matmuls lag the raw projections
                # by two chunks so PE never stalls on the ACT psum->sbuf copy
                qk_cols = [(0, 128), (128, 256), (256, 384)]
                chunks = [(m, n) for n in range(4) for m in range(3)]
                raws = {}

                def emit_raw(i):
                    m, n = chunks[i]
                    c0, c1 = qk_cols[m]
                    rows = 128
                    tsl = slice(512 * n, 512 * (n + 1))
                    praw = qps.tile([128, 512], f32, tag="ps", name=f"praw{i}")
                    for c in range(NCC):
                        nc.tensor.matmul(
                            praw[0:rows, :], wA[c][:, c0:c1], xT[c][:, tsl],
                            start=(c == 0), stop=(c == NCC - 1))
                    raw = qp.tile([128, 512], f32r, tag="raw", bufs=5,
                                  name=f"raw{i}")
                    nc.scalar.copy(raw[0:rows, :], praw[0:rows, :])
                    raws[i] = raw

                def emit_rope(i):
                    m, n = chunks[i]
                    tsl = slice(512 * n, 512 * (n + 1))
                    raw = raws.pop(i)
                    prot = qps.tile([128, 512], f32, tag="ps", name=f"prot{i}")
                    nc.tensor.matmul(prot[:], rT[:], raw[:], start=True,
                                     stop=True)
                    t1 = qp.tile([128, 512], f32, tag="t1", bufs=3,
                                 name=f"t1_{i}")
                    nc.gpsimd.tensor_tensor(t1[:], raw[:], cosT[:, tsl], mult)
                    t2 = qp.tile([128, 512], f32, tag="t2", bufs=3,
                                 name=f"t2_{i}")
                    nc.vector.tensor_tensor(t2[:], prot[:], sinT[:, tsl], mult)
                    if m < 2:
                        nc.vector.tensor_tensor(qkT[m][:, tsl], t1[:], t2[:],
                                                add)
                    else:
                        # packed [q2|k2]: split to qkT[2]/qkT[3] (cross-part)
                        nc.vector.tensor_tensor(qkT[2][:, tsl], t1[0:64, :],
                                                t2[0:64, :], add)
                        nc.vector.tensor_tensor(qkT[3][:, tsl], t1[64:128, :],
                                                t2[64:128, :], add)

                for i in range(len(chunks)):
                    emit_raw(i)
                    if i >= 2:
                        emit_rope(i - 2)
                for i in (len(chunks) - 2, len(chunks) - 1):
                    emit_rope(i)

                # V projection in natural [t, d] layout
                for t in range(TQ):
                    tsl = slice(128 * t, 128 * (t + 1))
                    pv = qps.tile([128, 256], f32, tag="ps", name=f"pv{t}")
                    for c in range(NCC):
                        nc.tensor.matmul(pv[:], xT[c][:, tsl],
                                         wA[c][:, 384:640], start=(c == 0),
                                         stop=(c == NCC - 1))
                    nc.vector.tensor_copy(
                        v_sb[:, t, :, 0:64],
                        pv[:, 0:192].rearrange("p (h d) -> p h d", d=64))

                qkv_ps_pool.__exit__(None, None, None)
                qkv_pool.__exit__(None, None, None)

                # ========== attention + out projection (interleaved) ==========
                attn_pool = tc.tile_pool(name=f"attn{rep}", bufs=1)
                ap = attn_pool.__enter__()
                attn_ps_pool = tc.tile_pool(name=f"attnps{rep}", bufs=2, space="PSUM")
                aps = attn_ps_pool.__enter__()

                # q/k row views per head: (tile index, partition offset)
                qv = [(0, 0), (0, 64), (2, 0)]
                kv = [(1, 0), (1, 64), (3, 0)]

                for g in range(NGRP):
                    for h in range(HPG):
                        qm, qo = qv[h]
                        km, ko = kv[h]
                        qT = qkT[qm][qo:qo + 64, :]
                        kT = qkT[km][ko:ko + 64, :]
                        nj = (GW // 128) * (g + 1)
                        # pass A: scores + exp (+ causal tri) for every k-chunk
                        pts = []
                        for j in range(nj):
                            dj = j - (GW // 128) * g
                            col0 = 128 * dj if dj >= 0 else 0
                            pscr = aps.tile([128, GW], f32, tag="pscr", bufs=2,
                                            name=f"pscr{g}_{h}_{j}")
                            for s0 in range(col0 - col0 % 512, GW, 512):
                                a0 = max(s0, col0)
                                nc.tensor.matmul(
                                    pscr[:, a0:s0 + 512],
                                    kT[:, 128 * j:128 * (j + 1)],
                                    qT[:, GW * g + a0:GW * g + s0 + 512],
                                    start=True, stop=True)
                            pt = ap.tile([128, GW], f32r, tag="pt", bufs=17,
                                         name=f"pt{g}_{h}_{j}")
                            nc.scalar.activation(pt[:, col0:], pscr[:, col0:],
                                                 Exp, scale=0.125)
                            if dj >= 0:
                                nc.gpsimd.tensor_tensor(
                                    pt[:, col0:col0 + 128],
                                    pt[:, col0:col0 + 128], tri[:], mult)
                            pts.append((pt, col0))
                        # pass B: P^T @ V into two 512-wide accumulators
                        pos = [aps.tile([65, 512], f32, tag="pso", bufs=4,
                                        name=f"po{g}_{h}_{i2}")
                               for i2 in range(GW // 512)]
                        lastw = {}
                        for j in range(nj):
                            _, col0 = pts[j]
                            for s0 in range(col0 - col0 % 512, GW, 512):
                                lastw[s0 // 512] = j
                        for j in range(nj):
                            pt, col0 = pts[j]
                            for s0 in range(col0 - col0 % 512, GW, 512):
                                a0 = max(s0, col0)
                                hv = s0 // 512
                                nc.tensor.matmul(
                                    pos[hv][:, a0 - s0:512], v_sb[:, j, h, :],
                                    pt[:, a0:s0 + 512], start=(j == 0),
                                    stop=(j == lastw[hv]), skip_group_check=True)
                        # normalize per half: attnT = po[0:64] * (1/po[64]),
                        # denominator broadcast across partitions on GPSIMD
                        for hv in range(GW // 512):
                            po = pos[hv]
                            csl = slice(GW * g + 512 * hv, GW * g + 512 * (hv + 1))
                            rc0 = ap.tile([1, 512], f32, tag="rc0", bufs=2,
                                          name=f"rc0{g}_{h}_{hv}")
                            nc.vector.reciprocal(rc0[:], po[64:65, :])
                            pbb = ap.tile([64, 512], f32, tag="pbb", bufs=3,
                                          name=f"pbb{g}_{h}_{hv}")
                            nc.gpsimd.partition_broadcast(pbb[:], rc0[:])
                            dstT, dofs = attn_dst[h]
                            nc.vector.tensor_tensor(dstT[dofs:dofs + D, csl],
                                                    po[0:64, :], pbb[:], mult)

                    # out projection for this g's t-range, from the same pool
                    for t in range((TQ // NGRP) * g, (TQ // NGRP) * (g + 1)):
                        tsl = slice(128 * t, 128 * (t + 1))
                        for c0, cn in ((0, 512), (512, 256)):
                            pout = aps.tile([128, cn], f32, tag="pso", bufs=4,
                                            name=f"pout{t}_{c0}")
                            nc.tensor.matmul(pout[:], attnT_A[:, tsl],
                                             woA[:, c0:c0 + cn], start=True,
                                             stop=False)
                            nc.tensor.matmul(pout[:], attnT_B[:, tsl],
                                             woB[:, c0:c0 + cn], start=False,
                                             stop=True)
                            osb = ap.tile([128, cn], f32, tag=f"osb{c0}", bufs=3,
                                          name=f"osb{t}_{c0}")
                            nc.any.tensor_copy(osb[:], pout[:])
                            dmaq[2 * ((t + (1 if c0 else 0)) % 2)].dma_start(
                                out_d[tsl, c0:c0 + cn], osb[:])

                attn_ps_pool.__exit__(None, None, None)
                attn_pool.__exit__(None, None, None)

    nc.compile()
    return nc


def _host_inputs(x, w_qkv, w_out):
    """Build the 8 per-core input maps."""
    inv_freq = 1.0 / (ROPE_BASE ** (np.arange(0, D, 2, dtype=np.float32) / D))
    t = np.arange(T, dtype=np.float32)
    freqs = t[:, None] * inv_freq[None, :]          # [T, D/2]
    emb = np.concatenate([freqs, freqs], axis=-1)   # [T, D]
    cosT = np.ascontiguousarray(np.cos(emb).T.astype(np.float32))  # [D, T]
    sinT = np.ascontiguousarray(np.sin(emb).T.astype(np.float32))
    cosT2 = np.concatenate([cosT, cosT], axis=0)    # [128, T]
    sinT2 = np.concatenate([sinT, sinT], axis=0)

    # rotate_half permutation as matmul lhsT: rot = R @ q, lhsT = R.T
    R = np.zeros((D, D), np.float32)
    R[0:32, 32:64] = -np.eye(32)
    R[32:64, 0:32] = np.eye(32)
    R2 = np.zeros((128, 128), np.float32)
    R2[0:64, 0:64] = R
    R2[64:128, 64:128] = R
    rT = np.ascontiguousarray(R2.T)

    tri = np.zeros((128, 128), np.float32)
    for kr in range(128):
        tri[kr, kr:] = 1.0

    wq = w_qkv[0:C]
    wk = w_qkv[C:2 * C]
    wv = w_qkv[2 * C:3 * C]

    maps = []
    for core in range(NG):
        b, hg = core // 4, core % 4
        hs = slice(HPG * D * hg, HPG * D * (hg + 1))   # 192 rows of this group
        h2 = HPG * D * hg + 2 * D
        q01 = wq[hs][0:128]                             # [128, C]
        k01 = wk[hs][0:128]
        q2 = wq[h2:h2 + D]
        k2 = wk[h2:h2 + D]
        v3 = wv[hs]                                     # [192, C]
        wA = np.zeros((C, 640), np.float32)
        wA[:, 0:128] = q01.T
        wA[:, 128:256] = k01.T
        wA[:, 256:320] = q2.T
        wA[:, 320:384] = k2.T
        wA[:, 384:576] = v3.T
        wo_h = [w_out[:, HPG * D * hg + D * h: HPG * D * hg + D * (h + 1)].T
                for h in range(HPG)]                    # 3 x [64, C]
        woA = np.concatenate([wo_h[0], wo_h[1]], axis=0)  # [128, C]
        woB = wo_h[2]                                     # [64, C]
        maps.append({
            "xT": np.ascontiguousarray(x[b].T),
            "wA": np.ascontiguousarray(wA),
            "woA": np.ascontiguousarray(woA.astype(np.float32)),
            "woB": np.ascontiguousarray(woB.astype(np.float32)),
            "cosT": cosT2, "sinT": sinT2,
            "rT": rT, "tri": tri,
        })
    return maps


def kernel(x, w_qkv, w_out):
    from concourse.bass_utils import run_bass_kernel_spmd

    if "nc" not in _CACHE:
        _CACHE["nc"] = _build_nc()
    nc = _CACHE["nc"]

    maps = _host_inputs(np.asarray(x, np.float32),
                        np.asarray(w_qkv, np.float32),
                        np.asarray(w_out, np.float32))
    res = run_bass_kernel_spmd(nc, maps, core_ids=list(range(NG))).results
    parts = np.stack([r["out"] for r in res])           # [8, T, C]
    out = np.zeros((B, T, C), np.float32)
    for b in range(B):
        out[b] = parts[4 * b:4 * (b + 1)].sum(axis=0)
    return out



# revision 36
# speedup vs baseline: 1.0717x; 1.0717x over previous
"""Multi-head attention (12 heads, RoPE, causal SDPA) for Trainium2, 8 cores.

Sharding: batch (2) x head-group (4 groups of 3 heads). Each core computes,
for its (batch b, head-group hg): QKV projection for its 3 heads, RoPE,
causal attention, and a partial out-projection [T, C] restricted to its
heads' rows of w_out. The host sums the 4 head-group partials per batch.

All matmuls and SBUF-resident tensors are bf16 (PSUM accumulation stays
f32), which halves HBM traffic and SBUF footprint vs f32 and avoids the
fp32r small-tile penalty. Device-side layouts (T=2048, C=768, D=64/head):

  xT    [128, 6, 2048]  x[b].T by contraction chunk (c on partitions)
  wqk   [128, 6, 384]   lhsT weights per chunk: cols [q0|q1][k0|k1][q2|k2]
  wv    [128, 6, 192]   V weights as matmul rhs (3 heads)
  cosT  [128, 2048]     RoPE cos, stacked twice (64 d x 2)
  sinT  [128, 2048]     RoPE sin, stacked twice
  rT    [128, 128]      rotate_half as matmul lhsT (runs on PE)
  tri   [128, 128]      tri[kr, qc] = 1 if qc >= kr (causal keep-mask)

Attention: scores are computed transposed (S^T[k, q] = K Q^T) so softmax
exp lands in [k, q] layout with no max-subtraction (scores are O(1) by
construction). P@V runs in natural layout with P^T as the stationary
operand: out[q, 65] blocks at 65 cycles per 128x128 tile, where column 64
(an all-ones column appended to V) accumulates the softmax denominator for
free. Normalization is then a native per-partition divide. The normalized
attention output [q, d] is transposed back to [d, t] for the out-projection
with the DMA engines' XBAR transpose (14 ns/tile, off the compute engines).

Emission is software-pipelined for the in-order engines: scores for heads
0/1 of q-group 0 are emitted right after their weight chunks so the ACT
engine (exp is the second-busiest stream) starts early; the group-1 score
loops are interleaved with P@V and out-projection units so the PE has work
while exp catches up.
"""
import numpy as np

B, T, C, H, D = 2, 2048, 768, 12, 64
HPG = 3                    # heads per group
NG = B * (H // HPG)        # 8 cores
ROPE_BASE = 10000.0
TQ = T // 128              # 16 t-tiles
NCC = C // 128             # 6 contraction chunks
GW = 1024                  # attention q-group width
NGRP = T // GW             # 2 q-groups

_CACHE = {}


def _build_nc(reps=1):
    from concourse import bacc, tile, mybir

    f32 = mybir.dt.float32
    bf16 = mybir.dt.bfloat16
    Exp = mybir.ActivationFunctionType.Exp
    mult = mybir.AluOpType.mult
    add = mybir.AluOpType.add
    div = mybir.AluOpType.divide

    nc = bacc.Bacc("TRN2", target_bir_lowering=False, debug=False,
                   num_devices=NG)

    xT_d = nc.dram_tensor("xT", [C, T], bf16, kind="ExternalInput").ap()
    wqk_d = nc.dram_tensor("wqk", [C, 384], bf16, kind="ExternalInput").ap()
    wv_d = nc.dram_tensor("wv", [C, 192], bf16, kind="ExternalInput").ap()
    woA_d = nc.dram_tensor("woA", [2 * D, C], bf16, kind="ExternalInput").ap()
    woB_d = nc.dram_tensor("woB", [D, C], bf16, kind="ExternalInput").ap()
    cosT_d = nc.dram_tensor("cosT", [128, T], bf16, kind="ExternalInput").ap()
    sinT_d = nc.dram_tensor("sinT", [128, T], bf16, kind="ExternalInput").ap()
    rT_d = nc.dram_tensor("rT", [128, 128], bf16, kind="ExternalInput").ap()
    tri_d = nc.dram_tensor("tri", [128, 128], bf16, kind="ExternalInput").ap()
    out_d = nc.dram_tensor("out", [T, C], bf16, kind="ExternalOutput").ap()

    with tile.TileContext(nc) as tc:
      for rep in range(reps):
        with tc.tile_pool(name=f"persist{rep}", bufs=1) as pp:
            # ---- persistent tiles + constant loads ----
            # big loads on SP; small early ones on ACT (idle until exp)
            wqk = pp.tile([128, NCC, 384], bf16, tag="wqk")
            nc.sync.dma_start(
                wqk[:], wqk_d.rearrange("(c p) i -> p c i", p=128))
            xT = pp.tile([128, NCC, T], bf16, tag="xT")
            nc.scalar.dma_start(
                xT[:, 0:3, 0:512],
                xT_d[0:384, 0:512].rearrange("(c p) i -> p c i", p=128))
            nc.sync.dma_start(
                xT[:, 3:6, 0:512],
                xT_d[384:768, 0:512].rearrange("(c p) i -> p c i", p=128))
            for n in range(1, 4):
                nsl = slice(512 * n, 512 * (n + 1))
                nc.sync.dma_start(
                    xT[:, :, nsl],
                    xT_d[:, nsl].rearrange("(c p) i -> p c i", p=128))
            cosT = pp.tile([128, T], bf16, tag="cosT")
            nc.scalar.dma_start(cosT[:], cosT_d[:])
            sinT = pp.tile([128, T], bf16, tag="sinT")
            nc.scalar.dma_start(sinT[:], sinT_d[:])
            rT = pp.tile([128, 128], bf16, tag="rT")
            nc.scalar.dma_start(rT[:], rT_d[:])
            wv = pp.tile([128, NCC, 192], bf16, tag="wv")
            nc.sync.dma_start(wv[:], wv_d.rearrange("(c p) i -> p c i", p=128))
            tri = pp.tile([128, 128], bf16, tag="tri")
            nc.scalar.dma_start(tri[:], tri_d[:])
            woA = pp.tile([2 * D, C], bf16, tag="woA")
            nc.scalar.dma_start(woA[:], woA_d[:])
            woB = pp.tile([D, C], bf16, tag="woB")
            nc.scalar.dma_start(woB[:], woB_d[:])

            qk_rows = [128, 128, 64, 64]
            qkT = [pp.tile([qk_rows[m], T], bf16, tag=f"qkT{m}",
                           name=f"qkT{m}") for m in range(4)]
            v_sb = pp.tile([128, TQ, HPG, 65], bf16, tag="v_sb")
            nc.gpsimd.memset(v_sb[:, :, :, 64:65], 1.0)
            attn_sb = pp.tile([128, TQ, 256], bf16, tag="attn_sb")
            nc.gpsimd.memset(attn_sb[:, :, 192:256], 0.0)
            attnT = pp.tile([128, TQ * 2, 128], bf16, tag="attnT")

            wp = tc.tile_pool(name=f"work{rep}", bufs=1)
            wk = wp.__enter__()
            ps_pool = tc.tile_pool(name=f"ps{rep}", bufs=4, space="PSUM")
            psp = ps_pool.__enter__()     # <=512 f32: praw/prot/pv/pos/pout
            pscr_pool = tc.tile_pool(name=f"pscr{rep}", bufs=2, space="PSUM")
            pscrp = pscr_pool.__enter__()          # [128, 1024] score tiles

            # ---- QKV projection + RoPE (rotate-half on PE) ----
            raws = {}

            def emit_raw(m, n):
                nsl = slice(512 * n, 512 * (n + 1))
                praw = psp.tile([128, 512], f32, tag="ps", name=f"praw{m}_{n}")
                for c in range(NCC):
                    nc.tensor.matmul(
                        praw[:], wqk[:, c, 128 * m:128 * (m + 1)],
                        xT[:, c, nsl], start=(c == 0), stop=(c == NCC - 1))
                raw = wk.tile([128, 512], bf16, tag="raw", bufs=4,
                              name=f"raw{m}_{n}")
                if m < 2 and n == 1:
                    nc.scalar.copy(raw[:], praw[:])
                else:
                    nc.vector.tensor_copy(raw[:], praw[:])
                raws[(m, n)] = raw

            def emit_rope(m, n):
                nsl = slice(512 * n, 512 * (n + 1))
                raw = raws.pop((m, n))
                prot = psp.tile([128, 512], f32, tag="ps", name=f"prot{m}_{n}")
                nc.tensor.matmul(prot[:], rT[:], raw[:], start=True, stop=True)
                t2 = wk.tile([128, 512], bf16, tag="t2", bufs=3,
                             name=f"t2_{m}_{n}")
                nc.vector.tensor_tensor(t2[:], prot[:], sinT[:, nsl], mult)
                t1 = wk.tile([128, 512], bf16, tag="t1", bufs=3,
                             name=f"t1_{m}_{n}")
                nc.vector.tensor_tensor(t1[:], raw[:], cosT[:, nsl], mult)
                if m < 2:
                    nc.vector.tensor_tensor(qkT[m][:, nsl], t1[:], t2[:], add)
                else:
                    nc.vector.tensor_tensor(qkT[2][:, nsl], t1[0:64, :],
                                            t2[0:64, :], add)
                    nc.vector.tensor_tensor(qkT[3][:, nsl], t1[64:128, :],
                                            t2[64:128, :], add)

            def emit_qk(ms, ns):
                chunks = [(m, n) for n in ns for m in ms]
                for i, (m, n) in enumerate(chunks):
                    emit_raw(m, n)
                    if i >= 1:
                        emit_rope(*chunks[i - 1])
                emit_rope(*chunks[-1])

            def qk_units(ms, ns):
                """emit_qk split into filler-sized closures (one per chunk)."""
                chunks = [(m, n) for n in ns for m in ms]

                def unit(i):
                    m, n = chunks[i]
                    emit_raw(m, n)
                    if i >= 1:
                        emit_rope(*chunks[i - 1])
                    if i == len(chunks) - 1:
                        emit_rope(m, n)
                return [lambda i=i: unit(i) for i in range(len(chunks))]

            def emit_v(ts):
                for t in ts:
                    tsl = slice(128 * t, 128 * (t + 1))
                    pv = psp.tile([128, 192], f32, tag="ps", name=f"pv{t}")
                    for c in range(NCC):
                        nc.tensor.matmul(pv[:], xT[:, c, tsl], wv[:, c, :],
                                         start=(c == 0), stop=(c == NCC - 1))
                    nc.vector.tensor_copy(
                        v_sb[:, t, :, 0:64],
                        pv[:].rearrange("p (h d) -> p h d", d=64))

            # q/k row views per head: (tile index, partition offset)
            qv = [(0, 0), (0, 64), (2, 0)]
            kv = [(1, 0), (1, 64), (3, 0)]

            def emit_a_unit(g, h, j, pt):
                """Scores + exp + causal mask for one (head, k-block)."""
                qm, qo = qv[h]
                km, ko = kv[h]
                qT = qkT[qm][qo:qo + 64, :]
                kT = qkT[km][ko:ko + 64, :]
                dj = j - (GW // 128) * g
                col0 = 128 * dj if dj >= 0 else 0
                pscr = pscrp.tile([128, GW], f32, tag="pscr",
                                  name=f"pscr{g}_{h}_{j}")
                for s0 in range(col0 - col0 % 512, GW, 512):
                    a0 = max(s0, col0)
                    nc.tensor.matmul(
                        pscr[:, a0:s0 + 512],
                        kT[:, 128 * j:128 * (j + 1)],
                        qT[:, GW * g + a0:GW * g + s0 + 512],
                        start=True, stop=True)
                nc.scalar.activation(pt[:, j, col0:], pscr[:, col0:],
                                     Exp, scale=0.125)
                if dj >= 0:
                    nc.gpsimd.tensor_tensor(
                        pt[:, j, col0:col0 + 128],
                        pt[:, j, col0:col0 + 128], tri[:], mult)

            def alloc_pt(g):
                nj = (GW // 128) * (g + 1)
                return wk.tile([128, nj, GW], bf16, tag=f"ptg{g}", bufs=2,
                               name=f"pt{g}")

            def emit_b_unit(g, h, qcl, pt):
                """P^T-stationary PV + denominator + normalize (one q-chunk)."""
                qq = (GW // 128) * g + qcl      # global q-block
                pos = psp.tile([128, 65], f32, tag="ps",
                               name=f"pos{g}_{h}_{qcl}")
                for j in range(qq + 1):
                    nc.tensor.matmul(
                        pos[:], pt[:, j, 128 * qcl:128 * (qcl + 1)],
                        v_sb[:, j, h, :], start=(j == 0), stop=(j == qq))
                rden = wk.tile([128, 1], f32, tag="rden", bufs=4,
                               name=f"rden{g}_{h}_{qcl}")
                nc.vector.reciprocal(rden[:], pos[:, 64:65])
                nc.vector.tensor_scalar(
                    attn_sb[:, qq, 64 * h:64 * (h + 1)], pos[:, 0:64],
                    rden[:], None, mult)

            def emit_transposes(tq0, ntq):
                nc.sync.dma_start_transpose(
                    attnT[:, 2 * tq0:2 * (tq0 + ntq), :],
                    attn_sb[:, tq0:tq0 + ntq, :])

            osb_t = {}

            def emit_store(tq):
                nc.sync.dma_start(out_d[128 * tq:128 * (tq + 1), :],
                                  osb_t.pop(tq)[:])

            def emit_outproj(tq, store=True):
                osb = wk.tile([128, C], bf16, tag="osb", bufs=4,
                              name=f"osb{tq}")
                osb_t[tq] = osb
                for c0, cn in ((0, 512), (512, 256)):
                    pout = psp.tile([128, cn], f32, tag="ps",
                                    name=f"pout{tq}_{c0}")
                    nc.tensor.matmul(pout[:], attnT[:, 2 * tq, :],
                                     woA[:, c0:c0 + cn], start=True,
                                     stop=False)
                    nc.tensor.matmul(pout[:], attnT[0:64, 2 * tq + 1, :],
                                     woB[:, c0:c0 + cn], start=False,
                                     stop=True)
                    if tq < 8:
                        nc.vector.tensor_copy(osb[:, c0:c0 + cn], pout[:])
                    else:
                        nc.scalar.copy(osb[:, c0:c0 + cn], pout[:])
                if store:
                    emit_store(tq)

            def emit_a_head(g, h, pt, fillers=()):
                """Emit all scores of (g, h), interleaving filler units."""
                nj = (GW // 128) * (g + 1)
                fillers = list(fillers)
                done = 0
                for j in range(nj):
                    emit_a_unit(g, h, j, pt)
                    want = (j + 1) * len(fillers) // nj
                    while done < want:
                        fillers[done]()
                        done += 1

            # ---- emission schedule ----
            # The three g=1 score loops are the ACT (exp) backbone; all other
            # PE work rides inside them as fillers so neither engine starves.
            emit_qk((0, 1), (0, 1))            # q01/k01 for q-group 0
            pt00 = alloc_pt(0)
            emit_a_head(0, 0, pt00, fillers=qk_units((2,), (0, 1)))
            pt01 = alloc_pt(0)
            emit_a_head(0, 1, pt01, fillers=qk_units((0, 1, 2), (2, 3)))
            pt10 = alloc_pt(1)
            emit_a_head(1, 0, pt10,
                        fillers=[lambda t=t: emit_v((t,))
                                 for t in range(TQ)])
            pt11 = alloc_pt(1)
            emit_a_head(1, 1, pt11,
                        fillers=[lambda q=q: emit_b_unit(0, 0, q, pt00)
                                 for q in range(8)]
                        + [lambda q=q: emit_b_unit(1, 0, q, pt10)
                           for q in range(8)])
            pt02 = alloc_pt(0)
            emit_a_head(0, 2, pt02,
                        fillers=[lambda q=q: emit_b_unit(0, 1, q, pt01)
                                 for q in range(8)])

            def finish_tq(tq, g, qcl, pt):
                emit_b_unit(g, 2, qcl, pt)      # last head for this q-chunk
                emit_transposes(tq, 1)
                emit_outproj(tq)

            fillers12 = []
            for q in range(8):
                fillers12.append(lambda q=q: emit_b_unit(1, 1, q, pt11))
                fillers12.append(lambda q=q: finish_tq(q, 0, q, pt02))
            pt12 = alloc_pt(1)
            emit_a_head(1, 2, pt12, fillers=fillers12)
            # tail: per q-chunk, PV of last head -> transpose -> out-proj,
            # software-pipelined (lag 2) so PE covers the cross-engine
            # norm->transpose latency with the next chunks' PV work
            for qcl in range(8):
                emit_b_unit(1, 2, qcl, pt12)
                if qcl >= 2:
                    emit_transposes(8 + qcl - 2, 1)
                    emit_outproj(8 + qcl - 2, store=False)
                if qcl >= 4:
                    emit_store(8 + qcl - 4)
            for tq in (14, 15):
                emit_transposes(tq, 1)
                emit_outproj(tq, store=False)
            for tq in (12, 13, 14, 15):
                emit_store(tq)

            pscr_pool.__exit__(None, None, None)
            ps_pool.__exit__(None, None, None)
            wp.__exit__(None, None, None)

    nc.compile()
    return nc


def _host_inputs(x, w_qkv, w_out):
    """Build the 8 per-core input maps (all device tensors bf16)."""
    import ml_dtypes
    bf = ml_dtypes.bfloat16

    inv_freq = 1.0 / (ROPE_BASE ** (np.arange(0, D, 2, dtype=np.float64) / D))
    t = np.arange(T, dtype=np.float64)
    freqs = t[:, None] * inv_freq[None, :]          # [T, D/2]
    emb = np.concatenate([freqs, freqs], axis=-1)   # [T, D]
    cosT = np.ascontiguousarray(np.cos(emb).T.astype(np.float32))  # [D, T]
    sinT = np.ascontiguousarray(np.sin(emb).T.astype(np.float32))
    cosT2 = np.concatenate([cosT, cosT], axis=0).astype(bf)    # [128, T]
    sinT2 = np.concatenate([sinT, sinT], axis=0).astype(bf)

    # rotate_half permutation as matmul lhsT: rot = R @ q, lhsT = R.T
    R = np.zeros((D, D), np.float32)
    R[0:D // 2, D // 2:D] = -np.eye(D // 2)
    R[D // 2:D, 0:D // 2] = np.eye(D // 2)
    R2 = np.zeros((128, 128), np.float32)
    R2[0:64, 0:64] = R
    R2[64:128, 64:128] = R
    rT = np.ascontiguousarray(R2.T).astype(bf)

    tri = np.zeros((128, 128), np.float32)
    for kr in range(128):
        tri[kr, kr:] = 1.0
    tri = tri.astype(bf)

    wq = w_qkv[0:C]
    wk = w_qkv[C:2 * C]
    wv = w_qkv[2 * C:3 * C]

    maps = []
    for core in range(NG):
        b, hg = core // 4, core % 4
        hs = slice(HPG * D * hg, HPG * D * (hg + 1))   # 192 rows of this group
        h2 = HPG * D * hg + 2 * D
        q01 = wq[hs][0:128]                             # [128, C]
        k01 = wk[hs][0:128]
        q2 = wq[h2:h2 + D]
        k2 = wk[h2:h2 + D]
        v3 = wv[hs]                                     # [192, C]
        wqk_a = np.zeros((C, 384), np.float32)
        wqk_a[:, 0:128] = q01.T
        wqk_a[:, 128:256] = k01.T
        wqk_a[:, 256:320] = q2.T
        wqk_a[:, 320:384] = k2.T
        wv_a = np.ascontiguousarray(v3.T)               # [C, 192]
        wo_h = [w_out[:, HPG * D * hg + D * h: HPG * D * hg + D * (h + 1)].T
                for h in range(HPG)]                    # 3 x [64, C]
        woA = np.concatenate([wo_h[0], wo_h[1]], axis=0)  # [128, C]
        woB = wo_h[2]                                     # [64, C]
        maps.append({
            "xT": np.ascontiguousarray(x[b].T).astype(bf),
            "wqk": wqk_a.astype(bf),
            "wv": wv_a.astype(bf),
            "woA": np.ascontiguousarray(woA).astype(bf),
            "woB": np.ascontiguousarray(woB).astype(bf),
            "cosT": cosT2, "sinT": sinT2,
            "rT": rT, "tri": tri,
        })
    return maps


def kernel(x, w_qkv, w_out):
    from concourse.bass_utils import run_bass_kernel_spmd

    if "nc" not in _CACHE:
        _CACHE["nc"] = _build_nc()
    nc = _CACHE["nc"]

    maps = _host_inputs(np.asarray(x, np.float32),
                        np.asarray(w_qkv, np.float32),
                        np.asarray(w_out, np.float32))
    res = run_bass_kernel_spmd(nc, maps, core_ids=list(range(NG))).results
    parts = np.stack([np.asarray(r["out"], dtype=np.float32)
                      for r in res])                    # [8, T, C]
    out = np.zeros((B, T, C), np.float32)
    for b in range(B):
        out[b] = parts[4 * b:4 * (b + 1)].sum(axis=0)
    return out


# revision 38
# speedup vs baseline: 1.0756x; 1.0037x over previous
"""Multi-head attention (12 heads, RoPE, causal SDPA) for Trainium2, 8 cores.

Sharding: batch (2) x head-group (4 groups of 3 heads). Each core computes,
for its (batch b, head-group hg): QKV projection for its 3 heads, RoPE,
causal attention, and a partial out-projection [T, C] restricted to its
heads' rows of w_out. The host sums the 4 head-group partials per batch.

All matmuls and SBUF-resident tensors are bf16 (PSUM accumulation stays
f32), which halves HBM traffic and SBUF footprint vs f32 and avoids the
fp32r small-tile penalty. Device-side layouts (T=2048, C=768, D=64/head):

  xT    [128, 6, 2048]  x[b].T by contraction chunk (c on partitions)
  wqk   [128, 6, 384]   lhsT weights per chunk: cols [q0|q1][k0|k1][q2|k2]
  wv    [128, 6, 192]   V weights as matmul rhs (3 heads)
  cosT  [128, 2048]     RoPE cos, stacked twice (64 d x 2)
  sinT  [128, 2048]     RoPE sin, stacked twice
  rT    [128, 128]      rotate_half as matmul lhsT (runs on PE)
  tri   [128, 128]      tri[kr, qc] = 1 if qc >= kr (causal keep-mask)

Attention: scores are computed transposed (S^T[k, q] = K Q^T) so softmax
exp lands in [k, q] layout with no max-subtraction (scores are O(1) by
construction). P@V runs in natural layout with P^T as the stationary
operand: out[q, 65] blocks at 65 cycles per 128x128 tile, where column 64
(an all-ones column appended to V) accumulates the softmax denominator for
free. Normalization is then a native per-partition divide. The normalized
attention output [q, d] is transposed back to [d, t] for the out-projection
with the DMA engines' XBAR transpose (14 ns/tile, off the compute engines).

Emission is software-pipelined for the in-order engines: scores for heads
0/1 of q-group 0 are emitted right after their weight chunks so the ACT
engine (exp is the second-busiest stream) starts early; the group-1 score
loops are interleaved with P@V and out-projection units so the PE has work
while exp catches up.
"""
import numpy as np

B, T, C, H, D = 2, 2048, 768, 12, 64
HPG = 3                    # heads per group
NG = B * (H // HPG)        # 8 cores
ROPE_BASE = 10000.0
TQ = T // 128              # 16 t-tiles
NCC = C // 128             # 6 contraction chunks
GW = 1024                  # attention q-group width
NGRP = T // GW             # 2 q-groups

_CACHE = {}


def _build_nc(reps=1):
    from concourse import bacc, tile, mybir

    f32 = mybir.dt.float32
    bf16 = mybir.dt.bfloat16
    Exp = mybir.ActivationFunctionType.Exp
    mult = mybir.AluOpType.mult
    add = mybir.AluOpType.add
    div = mybir.AluOpType.divide

    nc = bacc.Bacc("TRN2", target_bir_lowering=False, debug=False,
                   num_devices=NG)

    xT_d = nc.dram_tensor("xT", [C, T], bf16, kind="ExternalInput").ap()
    wqk_d = nc.dram_tensor("wqk", [C, 384], bf16, kind="ExternalInput").ap()
    wv_d = nc.dram_tensor("wv", [C, 192], bf16, kind="ExternalInput").ap()
    woA_d = nc.dram_tensor("woA", [2 * D, C], bf16, kind="ExternalInput").ap()
    woB_d = nc.dram_tensor("woB", [D, C], bf16, kind="ExternalInput").ap()
    cosT_d = nc.dram_tensor("cosT", [128, T], bf16, kind="ExternalInput").ap()
    sinT_d = nc.dram_tensor("sinT", [128, T], bf16, kind="ExternalInput").ap()
    rT_d = nc.dram_tensor("rT", [128, 128], bf16, kind="ExternalInput").ap()
    tri_d = nc.dram_tensor("tri", [128, 128], bf16, kind="ExternalInput").ap()
    out_d = nc.dram_tensor("out", [T, C], bf16, kind="ExternalOutput").ap()

    with tile.TileContext(nc) as tc:
      for rep in range(reps):
        with tc.tile_pool(name=f"persist{rep}", bufs=1) as pp:
            # ---- persistent tiles + constant loads ----
            # big loads on SP; small early ones on ACT (idle until exp)
            wqk = pp.tile([128, NCC, 384], bf16, tag="wqk")
            nc.sync.dma_start(
                wqk[:], wqk_d.rearrange("(c p) i -> p c i", p=128))
            xT = pp.tile([128, NCC, T], bf16, tag="xT")
            nc.scalar.dma_start(
                xT[:, 0:3, 0:512],
                xT_d[0:384, 0:512].rearrange("(c p) i -> p c i", p=128))
            nc.sync.dma_start(
                xT[:, 3:6, 0:512],
                xT_d[384:768, 0:512].rearrange("(c p) i -> p c i", p=128))
            for n in range(1, 4):
                nsl = slice(512 * n, 512 * (n + 1))
                nc.sync.dma_start(
                    xT[:, :, nsl],
                    xT_d[:, nsl].rearrange("(c p) i -> p c i", p=128))
            cosT = pp.tile([128, T], bf16, tag="cosT")
            nc.scalar.dma_start(cosT[:], cosT_d[:])
            sinT = pp.tile([128, T], bf16, tag="sinT")
            nc.scalar.dma_start(sinT[:], sinT_d[:])
            rT = pp.tile([128, 128], bf16, tag="rT")
            nc.scalar.dma_start(rT[:], rT_d[:])
            wv = pp.tile([128, NCC, 192], bf16, tag="wv")
            nc.sync.dma_start(wv[:], wv_d.rearrange("(c p) i -> p c i", p=128))
            tri = pp.tile([128, 128], bf16, tag="tri")
            nc.scalar.dma_start(tri[:], tri_d[:])
            woA = pp.tile([2 * D, C], bf16, tag="woA")
            nc.scalar.dma_start(woA[:], woA_d[:])
            woB = pp.tile([D, C], bf16, tag="woB")
            nc.scalar.dma_start(woB[:], woB_d[:])

            qk_rows = [128, 128, 64, 64]
            qkT = [pp.tile([qk_rows[m], T], bf16, tag=f"qkT{m}",
                           name=f"qkT{m}") for m in range(4)]
            v_sb = pp.tile([128, TQ, HPG, 65], bf16, tag="v_sb")
            nc.gpsimd.memset(v_sb[:, :, :, 64:65], 1.0)
            attn_sb = pp.tile([128, TQ, 256], bf16, tag="attn_sb")
            nc.gpsimd.memset(attn_sb[:, :, 192:256], 0.0)
            attnT = pp.tile([128, TQ * 2, 128], bf16, tag="attnT")

            wp = tc.tile_pool(name=f"work{rep}", bufs=1)
            wk = wp.__enter__()
            ps_pool = tc.tile_pool(name=f"ps{rep}", bufs=4, space="PSUM")
            psp = ps_pool.__enter__()     # <=512 f32: praw/prot/pv/pos/pout
            pscr_pool = tc.tile_pool(name=f"pscr{rep}", bufs=2, space="PSUM")
            pscrp = pscr_pool.__enter__()          # [128, 1024] score tiles

            # ---- QKV projection + RoPE (rotate-half on PE) ----
            raws = {}

            def emit_raw(m, n):
                nsl = slice(512 * n, 512 * (n + 1))
                praw = psp.tile([128, 512], f32, tag="ps", name=f"praw{m}_{n}")
                for c in range(NCC):
                    nc.tensor.matmul(
                        praw[:], wqk[:, c, 128 * m:128 * (m + 1)],
                        xT[:, c, nsl], start=(c == 0), stop=(c == NCC - 1))
                raw = wk.tile([128, 512], bf16, tag="raw", bufs=4,
                              name=f"raw{m}_{n}")
                if m < 2 and n == 1:
                    nc.scalar.copy(raw[:], praw[:])
                else:
                    nc.vector.tensor_copy(raw[:], praw[:])
                raws[(m, n)] = raw

            def emit_rope(m, n):
                nsl = slice(512 * n, 512 * (n + 1))
                raw = raws.pop((m, n))
                prot = psp.tile([128, 512], f32, tag="ps", name=f"prot{m}_{n}")
                nc.tensor.matmul(prot[:], rT[:], raw[:], start=True, stop=True)
                t2 = wk.tile([128, 512], bf16, tag="t2", bufs=3,
                             name=f"t2_{m}_{n}")
                nc.vector.tensor_tensor(t2[:], prot[:], sinT[:, nsl], mult)
                t1 = wk.tile([128, 512], bf16, tag="t1", bufs=3,
                             name=f"t1_{m}_{n}")
                nc.vector.tensor_tensor(t1[:], raw[:], cosT[:, nsl], mult)
                if m < 2:
                    nc.vector.tensor_tensor(qkT[m][:, nsl], t1[:], t2[:], add)
                else:
                    nc.vector.tensor_tensor(qkT[2][:, nsl], t1[0:64, :],
                                            t2[0:64, :], add)
                    nc.vector.tensor_tensor(qkT[3][:, nsl], t1[64:128, :],
                                            t2[64:128, :], add)

            def emit_qk(ms, ns):
                chunks = [(m, n) for n in ns for m in ms]
                for i, (m, n) in enumerate(chunks):
                    emit_raw(m, n)
                    if i >= 1:
                        emit_rope(*chunks[i - 1])
                emit_rope(*chunks[-1])

            def qk_units(ms, ns):
                """emit_qk split into filler-sized closures (one per chunk)."""
                chunks = [(m, n) for n in ns for m in ms]

                def unit(i):
                    m, n = chunks[i]
                    emit_raw(m, n)
                    if i >= 1:
                        emit_rope(*chunks[i - 1])
                    if i == len(chunks) - 1:
                        emit_rope(m, n)
                return [lambda i=i: unit(i) for i in range(len(chunks))]

            def emit_v(ts):
                for t in ts:
                    tsl = slice(128 * t, 128 * (t + 1))
                    pv = psp.tile([128, 192], f32, tag="ps", name=f"pv{t}")
                    for c in range(NCC):
                        nc.tensor.matmul(pv[:], xT[:, c, tsl], wv[:, c, :],
                                         start=(c == 0), stop=(c == NCC - 1))
                    nc.vector.tensor_copy(
                        v_sb[:, t, :, 0:64],
                        pv[:].rearrange("p (h d) -> p h d", d=64))

            # q/k row views per head: (tile index, partition offset)
            qv = [(0, 0), (0, 64), (2, 0)]
            kv = [(1, 0), (1, 64), (3, 0)]

            def emit_a_unit(g, h, j, pt):
                """Scores + exp + causal mask for one (head, k-block)."""
                qm, qo = qv[h]
                km, ko = kv[h]
                qT = qkT[qm][qo:qo + 64, :]
                kT = qkT[km][ko:ko + 64, :]
                dj = j - (GW // 128) * g
                col0 = 128 * dj if dj >= 0 else 0
                pscr = pscrp.tile([128, GW], f32, tag="pscr",
                                  name=f"pscr{g}_{h}_{j}")
                for s0 in range(col0 - col0 % 512, GW, 512):
                    a0 = max(s0, col0)
                    nc.tensor.matmul(
                        pscr[:, a0:s0 + 512],
                        kT[:, 128 * j:128 * (j + 1)],
                        qT[:, GW * g + a0:GW * g + s0 + 512],
                        start=True, stop=True)
                nc.scalar.activation(pt[:, j, col0:], pscr[:, col0:],
                                     Exp, scale=0.125)
                if dj >= 0:
                    nc.gpsimd.tensor_tensor(
                        pt[:, j, col0:col0 + 128],
                        pt[:, j, col0:col0 + 128], tri[:], mult)

            def alloc_pt(g):
                nj = (GW // 128) * (g + 1)
                return wk.tile([128, nj, GW], bf16, tag=f"ptg{g}", bufs=2,
                               name=f"pt{g}")

            def emit_b_unit(g, h, qcl, pt):
                """P^T-stationary PV + denominator + normalize (one q-chunk)."""
                qq = (GW // 128) * g + qcl      # global q-block
                pos = psp.tile([128, 65], f32, tag="ps",
                               name=f"pos{g}_{h}_{qcl}")
                for j in range(qq + 1):
                    nc.tensor.matmul(
                        pos[:], pt[:, j, 128 * qcl:128 * (qcl + 1)],
                        v_sb[:, j, h, :], start=(j == 0), stop=(j == qq))
                rden = wk.tile([128, 1], f32, tag="rden", bufs=4,
                               name=f"rden{g}_{h}_{qcl}")
                nc.vector.reciprocal(rden[:], pos[:, 64:65])
                nc.vector.tensor_scalar(
                    attn_sb[:, qq, 64 * h:64 * (h + 1)], pos[:, 0:64],
                    rden[:], None, mult)

            def emit_transposes(tq0, ntq):
                nc.sync.dma_start_transpose(
                    attnT[:, 2 * tq0:2 * (tq0 + ntq), :],
                    attn_sb[:, tq0:tq0 + ntq, :])

            osb_t = {}

            def emit_store(tq):
                nc.sync.dma_start(out_d[128 * tq:128 * (tq + 1), :],
                                  osb_t.pop(tq)[:])

            def emit_outproj(tq, store=True):
                osb = wk.tile([128, C], bf16, tag="osb", bufs=4,
                              name=f"osb{tq}")
                osb_t[tq] = osb
                for c0, cn in ((0, 512), (512, 256)):
                    pout = psp.tile([128, cn], f32, tag="ps",
                                    name=f"pout{tq}_{c0}")
                    nc.tensor.matmul(pout[:], attnT[:, 2 * tq, :],
                                     woA[:, c0:c0 + cn], start=True,
                                     stop=False)
                    nc.tensor.matmul(pout[:], attnT[0:64, 2 * tq + 1, :],
                                     woB[:, c0:c0 + cn], start=False,
                                     stop=True)
                    if tq < 8:
                        nc.vector.tensor_copy(osb[:, c0:c0 + cn], pout[:])
                    else:
                        nc.scalar.copy(osb[:, c0:c0 + cn], pout[:])
                if store:
                    emit_store(tq)

            def emit_a_head(g, h, pt, fillers=()):
                """Emit all scores of (g, h), interleaving filler units."""
                nj = (GW // 128) * (g + 1)
                fillers = list(fillers)
                done = 0
                for j in range(nj):
                    emit_a_unit(g, h, j, pt)
                    want = (j + 1) * len(fillers) // nj
                    while done < want:
                        fillers[done]()
                        done += 1

            # ---- emission schedule ----
            # The three g=1 score loops are the ACT (exp) backbone; all other
            # PE work rides inside them as fillers so neither engine starves.
            emit_qk((0, 1), (0, 1))            # q01/k01 for q-group 0
            u23 = qk_units((0, 1, 2), (2, 3))  # q-group 1 columns
            pt00 = alloc_pt(0)
            emit_a_head(0, 0, pt00,
                        fillers=qk_units((2,), (0, 1)) + u23[:2])
            pt01 = alloc_pt(0)
            emit_a_head(0, 1, pt01, fillers=u23[2:])
            pt10 = alloc_pt(1)
            emit_a_head(1, 0, pt10,
                        fillers=[lambda t=t: emit_v((t,))
                                 for t in range(TQ)])
            pt11 = alloc_pt(1)
            emit_a_head(1, 1, pt11,
                        fillers=[lambda q=q: emit_b_unit(0, 0, q, pt00)
                                 for q in range(8)]
                        + [lambda q=q: emit_b_unit(1, 0, q, pt10)
                           for q in range(8)])
            pt02 = alloc_pt(0)
            emit_a_head(0, 2, pt02,
                        fillers=[lambda q=q: emit_b_unit(0, 1, q, pt01)
                                 for q in range(8)])

            def finish_tq(tq, g, qcl, pt):
                emit_b_unit(g, 2, qcl, pt)      # last head for this q-chunk
                emit_transposes(tq, 1)
                emit_outproj(tq)

            fillers12 = []
            for q in range(8):
                fillers12.append(lambda q=q: emit_b_unit(1, 1, q, pt11))
                fillers12.append(lambda q=q: finish_tq(q, 0, q, pt02))
            pt12 = alloc_pt(1)
            emit_a_head(1, 2, pt12, fillers=fillers12)
            # tail: per q-chunk, PV of last head -> transpose -> out-proj,
            # software-pipelined (lag 2) so PE covers the cross-engine
            # norm->transpose latency with the next chunks' PV work
            for qcl in range(8):
                emit_b_unit(1, 2, qcl, pt12)
                if qcl >= 2:
                    emit_transposes(8 + qcl - 2, 1)
                    emit_outproj(8 + qcl - 2, store=False)
                if qcl >= 4:
                    emit_store(8 + qcl - 4)
            for tq in (14, 15):
                emit_transposes(tq, 1)
                emit_outproj(tq, store=False)
            for tq in (12, 13, 14, 15):
                emit_store(tq)

            pscr_pool.__exit__(None, None, None)
            ps_pool.__exit__(None, None, None)
            wp.__exit__(None, None, None)

    nc.compile()
    return nc


def _host_inputs(x, w_qkv, w_out):
    """Build the 8 per-core input maps (all device tensors bf16)."""
    import ml_dtypes
    bf = ml_dtypes.bfloat16

    inv_freq = 1.0 / (ROPE_BASE ** (np.arange(0, D, 2, dtype=np.float64) / D))
    t = np.arange(T, dtype=np.float64)
    freqs = t[:, None] * inv_freq[None, :]          # [T, D/2]
    emb = np.concatenate([freqs, freqs], axis=-1)   # [T, D]
    cosT = np.ascontiguousarray(np.cos(emb).T.astype(np.float32))  # [D, T]
    sinT = np.ascontiguousarray(np.sin(emb).T.astype(np.float32))
    cosT2 = np.concatenate([cosT, cosT], axis=0).astype(bf)    # [128, T]
    sinT2 = np.concatenate([sinT, sinT], axis=0).astype(bf)

    # rotate_half permutation as matmul lhsT: rot = R @ q, lhsT = R.T
    R = np.zeros((D, D), np.float32)
    R[0:D // 2, D // 2:D] = -np.eye(D // 2)
    R[D // 2:D, 0:D // 2] = np.eye(D // 2)
    R2 = np.zeros((128, 128), np.float32)
    R2[0:64, 0:64] = R
    R2[64:128, 64:128] = R
    rT = np.ascontiguousarray(R2.T).astype(bf)

    tri = np.zeros((128, 128), np.float32)
    for kr in range(128):
        tri[kr, kr:] = 1.0
    tri = tri.astype(bf)

    wq = w_qkv[0:C]
    wk = w_qkv[C:2 * C]
    wv = w_qkv[2 * C:3 * C]

    maps = []
    for core in range(NG):
        b, hg = core // 4, core % 4
        hs = slice(HPG * D * hg, HPG * D * (hg + 1))   # 192 rows of this group
        h2 = HPG * D * hg + 2 * D
        q01 = wq[hs][0:128]                             # [128, C]
        k01 = wk[hs][0:128]
        q2 = wq[h2:h2 + D]
        k2 = wk[h2:h2 + D]
        v3 = wv[hs]                                     # [192, C]
        wqk_a = np.zeros((C, 384), np.float32)
        wqk_a[:, 0:128] = q01.T
        wqk_a[:, 128:256] = k01.T
        wqk_a[:, 256:320] = q2.T
        wqk_a[:, 320:384] = k2.T
        wv_a = np.ascontiguousarray(v3.T)               # [C, 192]
        wo_h = [w_out[:, HPG * D * hg + D * h: HPG * D * hg + D * (h + 1)].T
                for h in range(HPG)]                    # 3 x [64, C]
        woA = np.concatenate([wo_h[0], wo_h[1]], axis=0)  # [128, C]
        woB = wo_h[2]                                     # [64, C]
        maps.append({
            "xT": np.ascontiguousarray(x[b].T).astype(bf),
            "wqk": wqk_a.astype(bf),
            "wv": wv_a.astype(bf),
            "woA": np.ascontiguousarray(woA).astype(bf),
            "woB": np.ascontiguousarray(woB).astype(bf),
            "cosT": cosT2, "sinT": sinT2,
            "rT": rT, "tri": tri,
        })
    return maps


def kernel(x, w_qkv, w_out):
    from concourse.bass_utils import run_bass_kernel_spmd

    if "nc" not in _CACHE:
        _CACHE["nc"] = _build_nc()
    nc = _CACHE["nc"]

    maps = _host_inputs(np.asarray(x, np.float32),
                        np.asarray(w_qkv, np.float32),
                        np.asarray(w_out, np.float32))
    res = run_bass_kernel_spmd(nc, maps, core_ids=list(range(NG))).results
    parts = np.stack([np.asarray(r["out"], dtype=np.float32)
                      for r in res])                    # [8, T, C]
    out = np.zeros((B, T, C), np.float32)
    for b in range(B):
        out[b] = parts[4 * b:4 * (b + 1)].sum(axis=0)
    return out
